# revision 4
# baseline (speedup 1.0000x reference)
"""4D conv (K0=3 outer taps x 3x3x3 inner, pad 1, stride 1) on 8 TRN2 cores.

Sharding: data-parallel over (batch, D0). 2 batches x 24 frames = 48 output
frames -> 6 per core. Each core receives its 8 input frames (6 + 2 halo,
zero-padded at the edges) directly from the host, so no device collectives.

Per-core kernel v2: kd3 taps are contracted INSIDE the matmul via shifted
partition replicas.  SBUF x tiles hold 4 copies of the 32 input channels:
partition group g in {0,1,2} is shifted by g elements (the three kd3 taps),
group 3 is unshifted.

  - A-tile: rows 0-95 of the PE array, K=96 = 3 taps x 32ci.  One A-matmul
    covers a whole (k0, kd1, kd2) tap row.  21 A-slots cover rows 0..20.
  - B-tile: rows 96-127, K=32, single tap per slot.  18 B-slots cover the
    remaining taps 63..80.
  - 2x col tiling: two spatial chunks (N=507) in column halves (M=64 out
    channels each).

Each pair needs only TWO psum banks (A + B accumulators), so C=3 pairs
share every weight load (the LDWEIGHTS bubble amortizes 3x) and 2 banks
stay free for drain overlap.  Row-group partials are summed on DVE
(psum -> sbuf, 2 ops per pair), with the fp32 bias folded into the first
reduction op.  Matmuls run in bf16 with fp32 PSUM accumulation.
"""

import ml_dtypes
import numpy as np

import concourse.bass as bass
import concourse.mybir as mybir
import concourse.tile as tile
from concourse.bass_utils import run_bass_kernel_spmd

F32 = mybir.dt.float32
BF16 = mybir.dt.bfloat16

# Problem constants (hardcoded per contract)
B, CI, O, D = 2, 32, 64, 24
K0 = 3
D2P = D + 2              # padded d2/d3 = 26
PLANE = D2P * D2P        # 676
D1P = D + 3              # d1 padded to 27: +1 conv pad each side, +1 OOB slack row
FRAMES_IN = 8            # 6 output frames + 2 halo input frames per core
FRAMES_OUT = 6
NSLAB = 4                # d1 slabs per frame (6 output rows each)
ROWS_OUT = 6             # output d1 rows per slab
NCHUNK = 507             # matmul moving free size ( = 6*676/8 )
HALF = 3 * PLANE         # 2028 = 3 output d1 rows, one col-group half
XTILE = (ROWS_OUT + 2) * PLANE + 56  # 8 input d1 rows + 56 OOB slack = 5464
NSLOT = 21               # A-slots (tap rows 0..20); B-slots = 18 (taps 63..80)
NB = 18
CPAIRS = 3               # pairs sharing one weight load (psum: 3*2 banks + 2 spare)
N_CORES = 8
SHIFTS = (0, 1, 2, 0)    # partition-group kd3 shifts; group 3 feeds the B tile


def _build_nc(n_slabs=NSLAB, n_frames=FRAMES_OUT):
    nc = bass.Bass()
    x_h = nc.declare_dram_parameter("x", [CI, FRAMES_IN, D1P * PLANE], BF16, isOutput=False)
    w_h = nc.declare_dram_parameter("w", [128, NSLOT * 64], BF16, isOutput=False)
    b_h = nc.declare_dram_parameter("b", [128, 1], F32, isOutput=False)
    o_h = nc.declare_dram_parameter("out", [O, FRAMES_OUT, D * D * D], F32, isOutput=True)

    n_pairs = n_frames * 4

    with tile.TileContext(nc) as tc:
        with (
            tc.tile_pool(name="wpool", bufs=1) as wpool,
            tc.tile_pool(name="xpool", bufs=8) as xpool,
            tc.tile_pool(name="opool", bufs=3) as opool,
            tc.tile_pool(name="psum", bufs=8, space="PSUM") as ppool,
        ):
            wt = wpool.tile([128, NSLOT * 64], BF16)
            nc.sync.dma_start(out=wt[:], in_=w_h[:])
            bt = wpool.tile([128, 1], F32)
            nc.sync.dma_start(out=bt[:], in_=b_h[:])

            for s in range(n_slabs):
                x_tiles = {}
                osb_tiles = {}
                flushed = set()

                def get_x(fi, s=s, x_tiles=x_tiles):
                    if fi not in x_tiles:
                        t = xpool.tile([128, XTILE], BF16, tag="x")
                        # 4 partition-group replicas; groups 0-2 are shifted
                        # by the kd3 tap offset, group 3 unshifted (B tile).
                        # 2 free-dim halves per replica: 8 parallel DMA
                        # streams per tile for faster prefetch.
                        h = XTILE // 2
                        base = s * ROWS_OUT * PLANE
                        for g, sh in enumerate(SHIFTS):
                            src = x_h[:, fi, base + sh: base + sh + XTILE]
                            nc.sync.dma_start(out=t[32 * g: 32 * g + 32, :h],
                                              in_=src[:, :h])
                            nc.sync.dma_start(out=t[32 * g: 32 * g + 32, h:],
                                              in_=src[:, h:])
                        x_tiles[fi] = t
                    return x_tiles[fi]

                def get_osb(f, osb_tiles=osb_tiles):
                    if f not in osb_tiles:
                        osb_tiles[f] = opool.tile([128, 4 * NCHUNK], F32, tag="osb", name=f"osb{f}")
                    return osb_tiles[f]

                for rot in range(n_pairs // CPAIRS):
                    pairs = [divmod(rot * CPAIRS + i, 4) for i in range(CPAIRS)]
                    psA = [
                        ppool.tile([128, NCHUNK], F32, tag="acc", name=f"a{i}",
                                   padded_shape=[128, 512])
                        for i in range(CPAIRS)
                    ]
                    psB = [
                        ppool.tile([128, NCHUNK], F32, tag="acc", name=f"b{i}",
                                   padded_shape=[128, 512])
                        for i in range(CPAIRS)
                    ]
                    for slot in range(NSLOT):
                        lhsA = wt[0:96, slot * 64: slot * 64 + 64]
                        lhsB = wt[96:128, slot * 64: slot * 64 + 64]
                        k0a, kd1a, kd2a = np.unravel_index(slot, (3, 3, 3))
                        offA = kd1a * PLANE + kd2a * D2P
                        if slot < NB:
                            k0b, kd1b, kd2b, kd3b = np.unravel_index(
                                63 + slot, (3, 3, 3, 3))
                            offB = kd1b * PLANE + kd2b * D2P + kd3b
                        for i, (f, pj) in enumerate(pairs):
                            xa = get_x(f + k0a)
                            for cg in range(2):
                                base = cg * HALF + pj * NCHUNK + offA
                                nc.tensor.matmul(
                                    psA[i][64 * cg: 64 * cg + 64, :],
                                    lhsA,
                                    xa[0:96, base: base + NCHUNK],
                                    start=slot == 0,
                                    stop=slot == NSLOT - 1,
                                    tile_position=(0, 64 * cg),
                                    skip_group_check=True,
                                )
                            if slot < NB:
                                xb = get_x(f + k0b)
                                for cg in range(2):
                                    base = cg * HALF + pj * NCHUNK + offB
                                    nc.tensor.matmul(
                                        psB[i][64 * cg: 64 * cg + 64, :],
                                        lhsB,
                                        xb[96:128, base: base + NCHUNK],
                                        start=slot == 0,
                                        stop=slot == NB - 1,
                                        tile_position=(96, 64 * cg),
                                        skip_group_check=True,
                                    )
                    # reduce the 2 row-tile partials into SBUF on DVE; B
                    # stops 3 slots early so its op leads the pair's end
                    for i, (f, pj) in enumerate(pairs):
                        osl = get_osb(f)[:, pj * NCHUNK: (pj + 1) * NCHUNK]
                        nc.vector.tensor_scalar_add(osl, psB[i][:], bt[:])
                        nc.vector.tensor_tensor(osl, osl, psA[i][:], mybir.AluOpType.add)
                    # write out completed frames: both col halves, 3 valid
                    # d1 rows each (one DMA per d1 row: DMA APs only
                    # balance up to 3 dims)
                    for f, pj in pairs:
                        if pj == 3 and f not in flushed:
                            flushed.add(f)
                            osb = osb_tiles[f]
                            for cg in range(2):
                                for r in range(3):
                                    src = (
                                        osb[64 * cg: 64 * cg + 64, :]
                                        .rearrange("p (r c d) -> p r c d",
                                                   r=3, c=D2P, d=D2P)
                                        [:, r, :D, :D]
                                    )
                                    dst = (
                                        o_h[:, f, :]
                                        .rearrange("o (r c d) -> o r c d", r=D, c=D, d=D)
                                        [:, s * ROWS_OUT + 3 * cg + r, :, :]
                                    )
                                    nc.sync.dma_start(out=dst, in_=src)
    return nc


# Instruction kinds whose waits live outside the engine sync-wait struct
# (DGE descriptors / barrier machinery) — leave those untouched.
_NO_SPLIT = {"EventSemaphore", "SemaphoreOp", "Call"}


def _split_multiwait(nc):
    """Walrus codegen accepts at most ONE sync wait per engine compute
    instruction on TRN2; Tile freely emits several. Hoist excess waits onto
    same-engine NoOps inserted just before the instruction — per-engine
    program order makes this equivalent."""
    ctr = 0
    for blk in nc.m.functions[0].blocks:
        il = blk.instructions
        i = 0
        while i < len(il):
            inst = il[i]
            si = inst.sync_info
            if (
                si is not None
                and len(si.on_wait) > 1
                and inst.opcode not in _NO_SPLIT
            ):
                waits = list(si.on_wait)
                nops = []
                for w in waits[:-1]:
                    ctr += 1
                    nop = mybir.InstNoOp(
                        name=f"I-wsplit-{ctr}", engine=inst.engine, ins=[], outs=[]
                    )
                    nop.sync_info = mybir.SyncInfo(on_wait=[w], on_update=[])
                    nops.append(nop)
                inst.sync_info = mybir.SyncInfo(
                    on_wait=[waits[-1]], on_update=list(si.on_update)
                )
                il[i:i] = nops
                i += len(nops)
            i += 1


_NC = None


def _get_nc():
    global _NC
    if _NC is None:
        _NC = _build_nc()
        _split_multiwait(_NC)
    return _NC


def _prep_inputs(x, w, b):
    """Host-side shard + pack. Returns list of per-core input maps."""
    x = np.asarray(x, dtype=np.float32)
    w = np.asarray(w, dtype=np.float32)
    b = np.asarray(b, dtype=np.float32)
    # pad: d0 by 1/1, d1 by 1/2 (extra OOB slack row), d2/d3 by 1/1
    xp = np.pad(x, ((0, 0), (0, 0), (1, 1), (1, 2), (1, 1), (1, 1)))
    # The reference's `xp.reshape(B*D0p, Ci, ...)` scrambles (B, Ci, D0p):
    # conv "frame" j has channels = flat volumes [j*32, j*32+32) of the
    # (B, Ci, D0p)-ordered volume pool. Output frame o of batch b sums tap i
    # applied to frame (b*26 + o + i).
    flat3 = xp.reshape(B * CI * (D + 2), D1P * PLANE)
    # w -> [tap(81), ci, o]; tap t = (k0, kd1, kd2, kd3) row-major
    arr = w.transpose(0, 3, 4, 5, 2, 1).reshape(81, CI, O)
    wsb = np.zeros((128, NSLOT * 64), dtype=ml_dtypes.bfloat16)
    for r in range(NSLOT):          # A: tap rows 0..20, kd3 via K-stack
        for k in range(3):
            wsb[32 * k: 32 * k + 32, 64 * r: 64 * r + 64] = arr[3 * r + k]
    for sl in range(NB):            # B: taps 63..80 one per slot
        wsb[96:128, 64 * sl: 64 * sl + 64] = arr[63 + sl]
    bsb = np.ascontiguousarray(np.tile(b, 2).reshape(128, 1))
    in_maps = []
    for c in range(N_CORES):
        bi, fc = divmod(c, 4)
        j0 = bi * CI * (D + 2) + 32 * 6 * fc
        block = flat3[j0: j0 + 32 * FRAMES_IN]
        xc = np.ascontiguousarray(
            block.reshape(FRAMES_IN, CI, D1P * PLANE).transpose(1, 0, 2)
        ).astype(ml_dtypes.bfloat16)
        in_maps.append({"x": xc, "w": wsb, "b": bsb})
    return in_maps


def _assemble(results):
    out = np.empty((B, O, D, D, D, D), dtype=np.float32)
    for c in range(N_CORES):
        bi, fc = divmod(c, 4)
        r = results[c]["out"].reshape(O, FRAMES_OUT, D, D, D)
        out[bi, :, 6 * fc: 6 * fc + FRAMES_OUT] = r
    return out


def kernel(x, w, b):
    nc = _get_nc()
    in_maps = _prep_inputs(x, w, b)
    res = run_bass_kernel_spmd(nc, in_maps, list(range(N_CORES)))
    return _assemble(res.results)


# revision 8
# speedup vs baseline: 2.0474x; 2.0474x over previous
"""4D conv (K0=3 outer taps x 3x3x3 inner, pad 1, stride 1) on 8 TRN2 cores.

Sharding: data-parallel over (batch, D0). 2 batches x 24 frames = 48 output
frames -> 6 per core. Each core receives its 8 input frames (6 + 2 halo,
zero-padded at the edges) directly from the host, so no device collectives.

Per-core kernel v2: kd3 taps are contracted INSIDE the matmul via shifted
partition replicas.  SBUF x tiles hold 4 copies of the 32 input channels:
partition group g in {0,1,2} is shifted by g elements (the three kd3 taps),
group 3 is unshifted.

  - A-tile: rows 0-95 of the PE array, K=96 = 3 taps x 32ci.  One A-matmul
    covers a whole (k0, kd1, kd2) tap row.  21 A-slots cover rows 0..20.
  - B-tile: rows 96-127, K=32, single tap per slot.  18 B-slots cover the
    remaining taps 63..80.
  - 2x col tiling: two spatial chunks (N=507) in column halves (M=64 out
    channels each).

Each pair needs only TWO psum banks (A + B accumulators), so C=3 pairs
share every weight load (the LDWEIGHTS bubble amortizes 3x) and 2 banks
stay free for drain overlap.  Row-group partials are summed on DVE
(psum -> sbuf, 2 ops per pair), with the fp32 bias folded into the first
reduction op.  Matmuls run in bf16 with fp32 PSUM accumulation.
"""

import ml_dtypes
import numpy as np

import concourse.bass as bass
import concourse.mybir as mybir
import concourse.tile as tile
from concourse.bass_utils import run_bass_kernel_spmd

F32 = mybir.dt.float32
BF16 = mybir.dt.bfloat16

# Problem constants (hardcoded per contract)
B, CI, O, D = 2, 32, 64, 24
K0 = 3
D2P = D + 2              # padded d2/d3 = 26
PLANE = D2P * D2P        # 676
D1P = D + 3              # d1 padded to 27: +1 conv pad each side, +1 OOB slack row
FRAMES_IN = 8            # 6 output frames + 2 halo input frames per core
FRAMES_OUT = 6
NSLAB = 4                # d1 slabs per frame (6 output rows each)
ROWS_OUT = 6             # output d1 rows per slab
NCHUNK = 507             # matmul moving free size ( = 6*676/8 )
HALF = 3 * PLANE         # 2028 = 3 output d1 rows, one col-group half
XTILE = (ROWS_OUT + 2) * PLANE + 56  # 8 input d1 rows + 56 OOB slack = 5464
NSLOT = 21               # A-slots (tap rows 0..20); B-slots = 18 (taps 63..80)
NB = 18
CPAIRS = 3               # pairs sharing one weight load (psum: 3*2 banks + 2 spare)
N_CORES = 8
SHIFTS = (0, 1, 2, 0)    # partition-group kd3 shifts; group 3 feeds the B tile


def _build_nc(n_slabs=NSLAB, n_frames=FRAMES_OUT):
    nc = bass.Bass()
    x_h = nc.declare_dram_parameter("x", [CI, FRAMES_IN, D1P * PLANE], BF16, isOutput=False)
    w_h = nc.declare_dram_parameter("w", [128, NSLOT * 64], BF16, isOutput=False)
    b_h = nc.declare_dram_parameter("b", [128, 1], F32, isOutput=False)
    o_h = nc.declare_dram_parameter("out", [O, FRAMES_OUT, D * D * D], F32, isOutput=True)

    n_pairs = n_frames * 4

    with tile.TileContext(nc) as tc:
        with (
            tc.tile_pool(name="wpool", bufs=1) as wpool,
            tc.tile_pool(name="xpool", bufs=8) as xpool,
            tc.tile_pool(name="opool", bufs=3) as opool,
            tc.tile_pool(name="psum", bufs=8, space="PSUM") as ppool,
        ):
            wt = wpool.tile([128, NSLOT * 64], BF16)
            nc.sync.dma_start(out=wt[:], in_=w_h[:])
            bt = wpool.tile([128, 1], F32)
            nc.sync.dma_start(out=bt[:], in_=b_h[:])

            for s in range(n_slabs):
                x_tiles = {}
                osb_tiles = {}
                flushed = set()

                def get_x(fi, s=s, x_tiles=x_tiles):
                    if fi not in x_tiles:
                        t = xpool.tile([128, XTILE], BF16, tag="x")
                        # 4 partition-group replicas; groups 0-2 are shifted
                        # by the kd3 tap offset, group 3 unshifted (B tile).
                        # 2 free-dim halves per replica: 8 parallel DMA
                        # streams per tile for faster prefetch.
                        h = XTILE // 2
                        base = s * ROWS_OUT * PLANE
                        for g, sh in enumerate(SHIFTS):
                            src = x_h[:, fi, base + sh: base + sh + XTILE]
                            nc.sync.dma_start(out=t[32 * g: 32 * g + 32, :h],
                                              in_=src[:, :h])
                            nc.sync.dma_start(out=t[32 * g: 32 * g + 32, h:],
                                              in_=src[:, h:])
                        x_tiles[fi] = t
                    return x_tiles[fi]

                def get_osb(f, osb_tiles=osb_tiles):
                    if f not in osb_tiles:
                        osb_tiles[f] = opool.tile([128, 4 * NCHUNK], F32, tag="osb", name=f"osb{f}")
                    return osb_tiles[f]

                for rot in range(n_pairs // CPAIRS):
                    pairs = [divmod(rot * CPAIRS + i, 4) for i in range(CPAIRS)]
                    psA = [
                        ppool.tile([128, NCHUNK], F32, tag="acc", name=f"a{i}",
                                   padded_shape=[128, 512])
                        for i in range(CPAIRS)
                    ]
                    psB = [
                        ppool.tile([128, NCHUNK], F32, tag="acc", name=f"b{i}",
                                   padded_shape=[128, 512])
                        for i in range(CPAIRS)
                    ]
                    for slot in range(NSLOT):
                        lhsA = wt[0:96, slot * 64: slot * 64 + 64]
                        lhsB = wt[96:128, slot * 64: slot * 64 + 64]
                        k0a, kd1a, kd2a = np.unravel_index(slot, (3, 3, 3))
                        offA = kd1a * PLANE + kd2a * D2P
                        if slot < NB:
                            k0b, kd1b, kd2b, kd3b = np.unravel_index(
                                63 + slot, (3, 3, 3, 3))
                            offB = kd1b * PLANE + kd2b * D2P + kd3b
                        # same-tile matmuls consecutive: the post-pass drops
                        # the repeated per-pair LDWEIGHTS so pairs 1..C-1
                        # reuse the loaded weights and pipeline at stream rate
                        for i, (f, pj) in enumerate(pairs):
                            xa = get_x(f + k0a)
                            for cg in range(2):
                                base = cg * HALF + pj * NCHUNK + offA
                                nc.tensor.matmul(
                                    psA[i][64 * cg: 64 * cg + 64, :],
                                    lhsA,
                                    xa[0:96, base: base + NCHUNK],
                                    start=slot == 0,
                                    stop=slot == NSLOT - 1,
                                    tile_position=(0, 64 * cg),
                                    skip_group_check=True,
                                )
                        if slot < NB:
                            for i, (f, pj) in enumerate(pairs):
                                xb = get_x(f + k0b)
                                for cg in range(2):
                                    base = cg * HALF + pj * NCHUNK + offB
                                    nc.tensor.matmul(
                                        psB[i][64 * cg: 64 * cg + 64, :],
                                        lhsB,
                                        xb[96:128, base: base + NCHUNK],
                                        start=slot == 0,
                                        stop=slot == NB - 1,
                                        tile_position=(96, 64 * cg),
                                        skip_group_check=True,
                                    )
                    # reduce the 2 row-tile partials into SBUF on DVE; B
                    # stops 3 slots early so its op leads the pair's end
                    for i, (f, pj) in enumerate(pairs):
                        osl = get_osb(f)[:, pj * NCHUNK: (pj + 1) * NCHUNK]
                        nc.vector.tensor_scalar_add(osl, psB[i][:], bt[:])
                        nc.vector.tensor_tensor(osl, osl, psA[i][:], mybir.AluOpType.add)
                    # write out completed frames: both col halves, 3 valid
                    # d1 rows each (one DMA per d1 row: DMA APs only
                    # balance up to 3 dims)
                    for f, pj in pairs:
                        if pj == 3 and f not in flushed:
                            flushed.add(f)
                            osb = osb_tiles[f]
                            for cg in range(2):
                                for r in range(3):
                                    src = (
                                        osb[64 * cg: 64 * cg + 64, :]
                                        .rearrange("p (r c d) -> p r c d",
                                                   r=3, c=D2P, d=D2P)
                                        [:, r, :D, :D]
                                    )
                                    dst = (
                                        o_h[:, f, :]
                                        .rearrange("o (r c d) -> o r c d", r=D, c=D, d=D)
                                        [:, s * ROWS_OUT + 3 * cg + r, :, :]
                                    )
                                    nc.sync.dma_start(out=dst, in_=src)
    return nc


def _ap_key(ap):
    """Stable identity for a lowered weights AP (offset + pattern)."""
    return (getattr(ap, "offset", None), str(getattr(ap, "ap", ap)))


def _fix_tile_sizes(nc):
    """bass rounds the K=96 A-matmul tile_size up to (128, 64), whose row
    mask covers the B tile's strip and serializes the two tiles.  Restore
    the actual 96-row extent so A uses strips {0,1,2} and B strip {3}."""
    n = 0
    for blk in nc.m.functions[0].blocks:
        for inst in blk.instructions:
            if (
                isinstance(inst, mybir.InstMatmult)
                and inst.tile_size == (128, 64)
            ):
                inst.tile_size = (96, 64)
                n += 1
    return n


def _dedupe_ldweights(nc):
    """Drop an InstLdweights when the same weights are already loaded at the
    same tile position (the final NEFF preserves this emission order, and
    MATMULs use whatever the array holds).  Only sync-free loads are
    eligible — a load carrying waits/updates must stay for its semaphores."""
    removed = 0
    for blk in nc.m.functions[0].blocks:
        state = {}
        keep = []
        for inst in blk.instructions:
            if isinstance(inst, mybir.InstLdweights):
                key = (inst.tile_position, _ap_key(inst.ins[0]))
                si = inst.sync_info
                clean = si is None or (not si.on_wait and not si.on_update)
                if state.get(inst.tile_position) == key and clean:
                    removed += 1
                    continue
                state[inst.tile_position] = key
            keep.append(inst)
        blk.instructions[:] = keep
    return removed


# Instruction kinds whose waits live outside the engine sync-wait struct
# (DGE descriptors / barrier machinery) — leave those untouched.
_NO_SPLIT = {"EventSemaphore", "SemaphoreOp", "Call"}


def _split_multiwait(nc):
    """Walrus codegen accepts at most ONE sync wait per engine compute
    instruction on TRN2; Tile freely emits several. Hoist excess waits onto
    same-engine NoOps inserted just before the instruction — per-engine
    program order makes this equivalent."""
    ctr = 0
    for blk in nc.m.functions[0].blocks:
        il = blk.instructions
        i = 0
        while i < len(il):
            inst = il[i]
            si = inst.sync_info
            if (
                si is not None
                and len(si.on_wait) > 1
                and inst.opcode not in _NO_SPLIT
            ):
                waits = list(si.on_wait)
                nops = []
                for w in waits[:-1]:
                    ctr += 1
                    nop = mybir.InstNoOp(
                        name=f"I-wsplit-{ctr}", engine=inst.engine, ins=[], outs=[]
                    )
                    nop.sync_info = mybir.SyncInfo(on_wait=[w], on_update=[])
                    nops.append(nop)
                inst.sync_info = mybir.SyncInfo(
                    on_wait=[waits[-1]], on_update=list(si.on_update)
                )
                il[i:i] = nops
                i += len(nops)
            i += 1


_NC = None


def _get_nc():
    global _NC
    if _NC is None:
        _NC = _build_nc()
        _dedupe_ldweights(_NC)
        _split_multiwait(_NC)
    return _NC


def _prep_inputs(x, w, b):
    """Host-side shard + pack. Returns list of per-core input maps."""
    x = np.asarray(x, dtype=np.float32)
    w = np.asarray(w, dtype=np.float32)
    b = np.asarray(b, dtype=np.float32)
    # pad: d0 by 1/1, d1 by 1/2 (extra OOB slack row), d2/d3 by 1/1
    xp = np.pad(x, ((0, 0), (0, 0), (1, 1), (1, 2), (1, 1), (1, 1)))
    # The reference's `xp.reshape(B*D0p, Ci, ...)` scrambles (B, Ci, D0p):
    # conv "frame" j has channels = flat volumes [j*32, j*32+32) of the
    # (B, Ci, D0p)-ordered volume pool. Output frame o of batch b sums tap i
    # applied to frame (b*26 + o + i).
    flat3 = xp.reshape(B * CI * (D + 2), D1P * PLANE)
    # w -> [tap(81), ci, o]; tap t = (k0, kd1, kd2, kd3) row-major
    arr = w.transpose(0, 3, 4, 5, 2, 1).reshape(81, CI, O)
    wsb = np.zeros((128, NSLOT * 64), dtype=ml_dtypes.bfloat16)
    for r in range(NSLOT):          # A: tap rows 0..20, kd3 via K-stack
        for k in range(3):
            wsb[32 * k: 32 * k + 32, 64 * r: 64 * r + 64] = arr[3 * r + k]
    for sl in range(NB):            # B: taps 63..80 one per slot
        wsb[96:128, 64 * sl: 64 * sl + 64] = arr[63 + sl]
    bsb = np.ascontiguousarray(np.tile(b, 2).reshape(128, 1))
    in_maps = []
    for c in range(N_CORES):
        bi, fc = divmod(c, 4)
        j0 = bi * CI * (D + 2) + 32 * 6 * fc
        block = flat3[j0: j0 + 32 * FRAMES_IN]
        xc = np.ascontiguousarray(
            block.reshape(FRAMES_IN, CI, D1P * PLANE).transpose(1, 0, 2)
        ).astype(ml_dtypes.bfloat16)
        in_maps.append({"x": xc, "w": wsb, "b": bsb})
    return in_maps


def _assemble(results):
    out = np.empty((B, O, D, D, D, D), dtype=np.float32)
    for c in range(N_CORES):
        bi, fc = divmod(c, 4)
        r = results[c]["out"].reshape(O, FRAMES_OUT, D, D, D)
        out[bi, :, 6 * fc: 6 * fc + FRAMES_OUT] = r
    return out


def kernel(x, w, b):
    nc = _get_nc()
    in_maps = _prep_inputs(x, w, b)
    res = run_bass_kernel_spmd(nc, in_maps, list(range(N_CORES)))
    return _assemble(res.results)
